# revision 5
# baseline (speedup 1.0000x reference)
"""Windowed attention w/ ring-buffer KV cache for TRN2, 8 NeuronCores.

Problem (hardcoded): B=1, S=1024 new tokens, H=16 heads, D=64,
cache C=10240, window W=8192, START_FRAME=9728.

Math (derived from the reference ring-buffer update; the updated cache is
not returned, only the attention output):
  wk = concat(cache_k[2560:9728], rope(k)),  wv = concat(cache_v[2560:9728], v)
  out = softmax(rope(q) @ wk^T / 8) @ wv     (non-causal, all 8192 keys)

Sharding: head-parallel, 2 heads per core (core c owns heads 2c, 2c+1).
Each core computes its full [1024, 2, 64] output slice; host concatenates.

Per-core kernel layout:
  - KT  [128, 8192] : K^T, partitions = (head(2) x d(64)), cols = window pos
  - QT  [128, 1024] : Q^T, same partition layout
  - Vp  [128, 64kb, 65] per head : V natural layout + ones column (col 64)
  - QK  : S^T[k,q] tiles via row-tiled matmuls (head0 rows 0-63, head1 64-127)
  - exp : ScalarE activation over 3-bank PSUM batches (scale=1/8 fused)
  - PV  : accumulate V'^T @ P^T into PSUM [65, 512] per (head, q-tile);
          row 64 = softmax denominator (from the ones column)
  - tail: PE-transpose [65,128] -> [128,65], reciprocal + scale on DVE, DMA out
"""

import numpy as np

H, D = 16, 64
S = 1024
W = 8192
OLD = 7168          # window rows taken from old cache (cache rows 2560:9728)
CLO, CHI = 2560, 9728
START = 9728
NCORES = 8
NKB = W // 128      # 64
OLDKB = OLD // 128  # 56
NEWKB = S // 128    # 8
NQT = S // 128      # 8 token blocks
SCALE = 0.125

_cache = {}


def _build(niters=1):
    import concourse.mybir as mybir
    import concourse.tile as tile
    from concourse import bacc
    from concourse._compat import axon_active
    from concourse.bass import ds
    from concourse.masks import make_identity

    dt = mybir.dt.float32
    AF = mybir.ActivationFunctionType
    ALU = mybir.AluOpType

    nc = bacc.Bacc(
        "TRN2", target_bir_lowering=False, debug=not axon_active(),
        num_devices=NCORES,
    )
    kt_old = nc.dram_tensor("kt_old", [128, OLD], dt, kind="ExternalInput")
    v_old = nc.dram_tensor("v_old", [2, 128, OLDKB, D], dt, kind="ExternalInput")
    qn = nc.dram_tensor("qn", [S, 128], dt, kind="ExternalInput")
    kn = nc.dram_tensor("kn", [S, 128], dt, kind="ExternalInput")
    vn = nc.dram_tensor("vn", [2, 128, NEWKB, D], dt, kind="ExternalInput")
    cos2 = nc.dram_tensor("cos2", [S, D], dt, kind="ExternalInput")
    sin2 = nc.dram_tensor("sin2", [S, D], dt, kind="ExternalInput")
    out = nc.dram_tensor("out", [S, 2, D], dt, kind="ExternalOutput")

    with tile.TileContext(nc) as tc:
        for _it in range(niters):
            _emit_body(nc, tc, mybir, ds, make_identity, dt, AF, ALU,
                       kt_old, v_old, qn, kn, vn, cos2, sin2, out)

    nc.compile()
    return nc


def _emit_body(nc, tc, mybir, ds, make_identity, dt, AF, ALU,
               kt_old, v_old, qn, kn, vn, cos2, sin2, out):
    if True:
        with tc.tile_pool(name="const", bufs=1) as constp, \
             tc.tile_pool(name="pers", bufs=1) as pers, \
             tc.tile_pool(name="wk", bufs=4) as wkp, \
             tc.tile_pool(name="ptp", bufs=3) as ptp, \
             tc.tile_pool(name="stp", bufs=2, space="PSUM") as stp, \
             tc.tile_pool(name="pvp", bufs=1, space="PSUM") as pvp, \
             tc.tile_pool(name="osbp", bufs=1) as osbp, \
             tc.tile_pool(name="finp", bufs=4) as finp:

            ident = constp.tile([128, 128], dt, name="ident", tag="ident")
            make_identity(nc, ident)

            KT = pers.tile([128, W], dt, name="KT", tag="KT")
            QT = pers.tile([128, S], dt, name="QT", tag="QT")
            Vp0 = pers.tile([128, NKB, 65], dt, name="Vp0", tag="Vp0")
            Vp1 = pers.tile([128, NKB, 65], dt, name="Vp1", tag="Vp1")
            Vt = [Vp0, Vp1]
            qn_sb = pers.tile([128, NQT, 128], dt, name="qn_sb", tag="qn_sb")
            kn_sb = pers.tile([128, NQT, 128], dt, name="kn_sb", tag="kn_sb")
            cos_sb = pers.tile([128, NQT, D], dt, name="cos_sb", tag="cos_sb")
            sin_sb = pers.tile([128, NQT, D], dt, name="sin_sb", tag="sin_sb")
            qr = pers.tile([128, NQT, 128], dt, name="qr", tag="qr")
            kr = pers.tile([128, NQT, 128], dt, name="kr", tag="kr")
            dume = pers.tile([128, 2], dt, name="dume", tag="dume")

            # preload the exp table set early (ACT is the critical engine)
            nc.vector.memset(dume[:, :], 0.0)
            nc.scalar.activation(dume[:, :], dume[:, :], AF.Exp)

            # ---- input DMAs ----
            for h in range(2):
                nc.sync.dma_start(Vt[h][:, 0:OLDKB, 0:64], v_old.ap()[h])
                nc.sync.dma_start(Vt[h][:, OLDKB:NKB, 0:64], vn.ap()[h])
                nc.vector.memset(Vt[h][:, :, 64], 1.0)
            for i in range(7):
                nc.sync.dma_start(KT[:, ds(i * 1024, 1024)],
                                  kt_old.ap()[:, ds(i * 1024, 1024)])
            nc.sync.dma_start(qn_sb[:, :, :],
                              qn.ap().rearrange("(n p) f -> p n f", p=128))
            nc.sync.dma_start(kn_sb[:, :, :],
                              kn.ap().rearrange("(n p) f -> p n f", p=128))
            nc.sync.dma_start(cos_sb[:, :, :],
                              cos2.ap().rearrange("(n p) d -> p n d", p=128))
            nc.sync.dma_start(sin_sb[:, :, :],
                              sin2.ap().rearrange("(n p) d -> p n d", p=128))

            # ---- RoPE (interleaved pairs) on DVE, all 8 token blocks/op ----
            cvw = cos_sb.rearrange("p n (h j) -> p n h j", h=2)
            svw = sin_sb.rearrange("p n (h j) -> p n h j", h=2)

            def rope(src, dst, pfx):
                x = src.rearrange("p n (h j two) -> p n h j two", h=2, two=2)
                o = dst.rearrange("p n (h j two) -> p n h j two", h=2, two=2)
                x1, x2 = x[:, :, :, :, 0], x[:, :, :, :, 1]
                o1, o2 = o[:, :, :, :, 0], o[:, :, :, :, 1]
                ta = wkp.tile([128, NQT, 2, 32], dt, tag="rt", bufs=4,
                              name=f"{pfx}ta")
                tb = wkp.tile([128, NQT, 2, 32], dt, tag="rt", bufs=4,
                              name=f"{pfx}tb")
                nc.vector.tensor_mul(ta[:], x1, cvw)
                nc.vector.tensor_mul(tb[:], x2, svw)
                nc.vector.tensor_sub(o1, ta[:], tb[:])
                tc2 = wkp.tile([128, NQT, 2, 32], dt, tag="rt", bufs=4,
                               name=f"{pfx}tc")
                td = wkp.tile([128, NQT, 2, 32], dt, tag="rt", bufs=4,
                               name=f"{pfx}td")
                nc.vector.tensor_mul(tc2[:], x2, cvw)
                nc.vector.tensor_mul(td[:], x1, svw)
                nc.vector.tensor_add(o2, tc2[:], td[:])

            rope(qn_sb, qr, "q")
            rope(kn_sb, kr, "k")

            # ---- transposes: roped [tok, (h d)] -> [(h d), tok] ----
            tp0 = stp.tile([128, 3, 512], dt, tag="st", bufs=2, name="tp0")
            tv0 = tp0.rearrange("p a b -> p (a b)")
            for n in range(NQT):
                slot = tv0[:, ds(n * 128, 128)]
                nc.tensor.transpose(slot, qr[:, n, :], ident[:, :])
                nc.vector.tensor_copy(QT[:, ds(n * 128, 128)], slot)
            tp1 = stp.tile([128, 3, 512], dt, tag="st", bufs=2, name="tp1")
            tv1 = tp1.rearrange("p a b -> p (a b)")
            for n in range(NQT):
                slot = tv1[:, ds(n * 128, 128)]
                nc.tensor.transpose(slot, kr[:, n, :], ident[:, :])
                nc.vector.tensor_copy(KT[:, ds(OLD + n * 128, 128)], slot)

            # ---- main loop: QK -> exp -> PV ----
            osb = {}
            for qt in range(2):
                qsl = ds(qt * 512, 512)
                pvt = [pvp.tile([65, 512], dt, tag=f"pv{h}", bufs=1,
                                name=f"pv{qt}{h}") for h in range(2)]
                slices = [(kb, h) for kb in range(NKB) for h in range(2)]
                for b0 in range(0, len(slices), 3):
                    batch = slices[b0:b0 + 3]
                    nb = len(batch)
                    st = stp.tile([128, 3, 512], dt, tag="st", bufs=2,
                                  name=f"st{qt}_{b0}")
                    for i, (kb, h) in enumerate(batch):
                        nc.tensor.matmul(
                            st[:, i, :],
                            lhsT=KT[64 * h:64 * h + 64, ds(kb * 128, 128)],
                            rhs=QT[64 * h:64 * h + 64, qsl],
                            start=True, stop=True,
                            tile_position=(64 * h, 0),
                        )
                    pt = ptp.tile([128, 3, 512], dt, tag="pt", bufs=3,
                                  name=f"pt{qt}_{b0}")
                    nc.scalar.activation(pt[:, 0:nb, :], st[:, 0:nb, :],
                                         AF.Exp, scale=SCALE)
                    for i, (kb, h) in enumerate(batch):
                        nc.tensor.matmul(
                            pvt[h],
                            lhsT=Vt[h][:, kb, :],
                            rhs=pt[:, i, :],
                            start=(kb == 0), stop=(kb == NKB - 1),
                        )
                for h in range(2):
                    ot = osbp.tile([65, 512], dt, tag=f"osb{qt}{h}", bufs=1,
                                   name=f"osb{qt}{h}")
                    nc.vector.tensor_copy(ot[:], pvt[h])
                    osb[(qt, h)] = ot

            # ---- tail: transpose, normalize, store ----
            for qt in range(2):
                for h in range(2):
                    ot = osb[(qt, h)]
                    tt = stp.tile([128, 3, 512], dt, tag="st", bufs=2,
                                  name=f"tt{qt}{h}")
                    ttv = tt.rearrange("p a b -> p (a b)")
                    for j in range(4):
                        slot = ttv[:, ds(j * 128, 128)][:, 0:65]
                        nc.tensor.transpose(slot, ot[:, ds(j * 128, 128)],
                                            ident[0:65, 0:65])
                        rec = finp.tile([128, 1], dt, tag="rec", bufs=4,
                                        name=f"rec{qt}{h}{j}")
                        nc.vector.reciprocal(rec[:], slot[:, 64:65])
                        fin = finp.tile([128, 64], dt, tag="fin", bufs=4,
                                        name=f"fin{qt}{h}{j}")
                        nc.vector.tensor_scalar(fin[:], slot[:, 0:64],
                                                rec[:], None, ALU.mult)
                        nc.sync.dma_start(
                            out.ap()[ds(qt * 512 + j * 128, 128), h, :],
                            fin[:])


def _prep_inputs(q, k, v, cache_k, cache_v, freqs_cos, freqs_sin):
    """Host-side sharding + layout prep (no FLOPs beyond data movement)."""
    q = np.asarray(q, np.float32)
    k = np.asarray(k, np.float32)
    v = np.asarray(v, np.float32)
    cache_k = np.asarray(cache_k, np.float32)
    cache_v = np.asarray(cache_v, np.float32)
    cos_h = np.asarray(freqs_cos, np.float32)[START:START + S, 0::2]
    sin_h = np.asarray(freqs_sin, np.float32)[START:START + S, 0::2]
    cos2 = np.ascontiguousarray(np.tile(cos_h, (1, 2)))
    sin2 = np.ascontiguousarray(np.tile(sin_h, (1, 2)))

    in_maps = []
    for c in range(NCORES):
        hs = slice(2 * c, 2 * c + 2)
        k_old = cache_k[0, CLO:CHI, hs, :]                      # [7168, 2, 64]
        kt_old = np.ascontiguousarray(
            k_old.transpose(1, 2, 0).reshape(128, OLD))
        v_old = np.ascontiguousarray(
            cache_v[0, CLO:CHI, hs, :].transpose(1, 0, 2)
            .reshape(2, OLDKB, 128, D).transpose(0, 2, 1, 3))   # [2,128,56,64]
        qn = np.ascontiguousarray(q[0, :, hs, :].reshape(S, 128))
        kn = np.ascontiguousarray(k[0, :, hs, :].reshape(S, 128))
        vn = np.ascontiguousarray(
            v[0, :, hs, :].transpose(1, 0, 2)
            .reshape(2, NEWKB, 128, D).transpose(0, 2, 1, 3))   # [2,128,8,64]
        in_maps.append({
            "kt_old": kt_old, "v_old": v_old, "qn": qn, "kn": kn,
            "vn": vn, "cos2": cos2, "sin2": sin2,
        })
    return in_maps


def get_nc(niters=1):
    key = ("nc", niters)
    if key not in _cache:
        _cache[key] = _build(niters)
    return _cache[key]


def _run(in_maps, niters=1):
    from concourse.bass_utils import run_bass_kernel_spmd
    res = run_bass_kernel_spmd(get_nc(niters), in_maps,
                               core_ids=list(range(NCORES)))
    out_full = np.empty((1, S, H, D), np.float32)
    for c in range(NCORES):
        out_full[0, :, 2 * c:2 * c + 2, :] = res.results[c]["out"].reshape(
            S, 2, D)
    return out_full.reshape(1, S, H * D), res


def kernel(q, k, v, cache_k, cache_v, freqs_cos, freqs_sin):
    in_maps = _prep_inputs(q, k, v, cache_k, cache_v, freqs_cos, freqs_sin)
    out, _ = _run(in_maps)
    return out


# revision 12
# speedup vs baseline: 1.6487x; 1.6487x over previous
"""Windowed attention w/ ring-buffer KV cache for TRN2, 8 NeuronCores.

Problem (hardcoded): B=1, S=1024 new tokens, H=16 heads, D=64,
cache C=10240, window W=8192, START_FRAME=9728.

Math (derived from the reference ring-buffer update; the updated cache is
not returned, only the attention output):
  wk = concat(cache_k[2560:9728], rope(k)),  wv = concat(cache_v[2560:9728], v)
  out = softmax(rope(q) @ wk^T / 8) @ wv     (non-causal, all 8192 keys)

Sharding: head-parallel, 2 heads per core (core c owns heads 2c, 2c+1).
Each core computes its full [1024, 2, 64] output slice; host concatenates.

Per-core kernel layout:
  - KT  [128, 8192] : K^T, partitions = (head(2) x d(64)), cols = window pos
  - QT  [128, 1024] : Q^T, same partition layout
  - Vp  [128, 64kb, 65] per head : V natural layout + ones column (col 64)
  - QK  : S^T[k,q] tiles via row-tiled matmuls (head0 rows 0-63, head1 64-127)
  - exp : ScalarE activation over 3-bank PSUM batches (scale=1/8 fused)
  - PV  : accumulate V'^T @ P^T into PSUM [65, 512] per (head, q-tile);
          row 64 = softmax denominator (from the ones column)
  - tail: PE-transpose [65,128] -> [128,65], reciprocal + scale on DVE, DMA out
"""

import numpy as np

H, D = 16, 64
S = 1024
W = 8192
OLD = 7168          # window rows taken from old cache (cache rows 2560:9728)
CLO, CHI = 2560, 9728
START = 9728
NCORES = 8
NKB = W // 128      # 64
OLDKB = OLD // 128  # 56
NEWKB = S // 128    # 8
NQT = S // 128      # 8 token blocks
SCALE = 0.125

_cache = {}


def _build(niters=1):
    import concourse.mybir as mybir
    import concourse.tile as tile
    from concourse import bacc
    from concourse._compat import axon_active
    from concourse.bass import ds
    from concourse.masks import make_identity

    dt = mybir.dt.float32
    dtr = mybir.dt.float32r
    AF = mybir.ActivationFunctionType
    ALU = mybir.AluOpType

    nc = bacc.Bacc(
        "TRN2", target_bir_lowering=False, debug=not axon_active(),
        num_devices=NCORES,
    )
    kt_old = nc.dram_tensor("kt_old", [128, OLD], dt, kind="ExternalInput")
    v_all = nc.dram_tensor("v_all", [2, 128, D, NKB], dt, kind="ExternalInput")
    qn = nc.dram_tensor("qn", [S, 128], dt, kind="ExternalInput")
    kn = nc.dram_tensor("kn", [S, 128], dt, kind="ExternalInput")
    cs = nc.dram_tensor("cs", [S, 128], dt, kind="ExternalInput")
    out = nc.dram_tensor("out", [S, 2, D], dt, kind="ExternalOutput")

    with tile.TileContext(nc) as tc:
        for _it in range(niters):
            _emit_body(nc, tc, mybir, ds, make_identity, dt, dtr, AF, ALU,
                       kt_old, v_all, qn, kn, cs, out)

    nc.compile()
    return nc


def _emit_body(nc, tc, mybir, ds, make_identity, dt, dtr, AF, ALU,
               kt_old, v_all, qn, kn, cs, out):
    if True:
        with tc.tile_pool(name="const", bufs=1) as constp, \
             tc.tile_pool(name="pers", bufs=1) as pers, \
             tc.tile_pool(name="wk", bufs=4) as wkp, \
             tc.tile_pool(name="ptp", bufs=3) as ptp, \
             tc.tile_pool(name="stp", bufs=2, space="PSUM") as stp, \
             tc.tile_pool(name="pvp", bufs=1, space="PSUM") as pvp, \
             tc.tile_pool(name="osbp", bufs=1) as osbp, \
             tc.tile_pool(name="finp", bufs=4) as finp:

            ident = constp.tile([128, 128], dt, name="ident", tag="ident")
            make_identity(nc, ident)

            KT = pers.tile([128, W], dtr, name="KT", tag="KT")
            QT = pers.tile([128, S], dtr, name="QT", tag="QT")
            Vp0 = pers.tile([128, 65, NKB], dtr, name="Vp0", tag="Vp0")
            Vp1 = pers.tile([128, 65, NKB], dtr, name="Vp1", tag="Vp1")
            Vt = [Vp0, Vp1]
            qn_sb = pers.tile([128, NQT, 128], dt, name="qn_sb", tag="qn_sb")
            kn_sb = pers.tile([128, NQT, 128], dt, name="kn_sb", tag="kn_sb")
            cs_sb = pers.tile([128, NQT, 128], dt, name="cs_sb", tag="cs_sb")
            cos_sb = cs_sb[:, :, 0:64]
            sin_sb = cs_sb[:, :, 64:128]
            qr = pers.tile([128, NQT, 128], dt, name="qr", tag="qr")
            kr = pers.tile([128, NQT, 128], dt, name="kr", tag="kr")
            dume = pers.tile([128, 2], dt, name="dume", tag="dume")

            # preload the exp table set early (ACT is the critical engine)
            nc.vector.memset(dume[:, :], 0.0)
            nc.scalar.activation(dume[:, :], dume[:, :], AF.Exp)

            # ---- input DMAs ----
            # Order matters: the rope chain (qn+cos+sin -> rope -> transpose
            # -> QK) is the critical lead-in path, so its feeds go first on
            # the sync queue. Bulk V goes on the gpsimd SWDGE queue; KT
            # chunks follow on sync in consumption (kb) order.
            nc.sync.dma_start(cs_sb[:, :, :],
                              cs.ap().rearrange("(n p) f -> p n f", p=128))
            nc.sync.dma_start(qn_sb[:, :, :],
                              qn.ap().rearrange("(n p) f -> p n f", p=128))
            nc.sync.dma_start(KT[:, ds(0, 1024)],
                              kt_old.ap()[:, ds(0, 1024)].bitcast(dtr))
            ones = wkp.tile([128, NKB], dt, tag="ones", bufs=1, name="ones")
            nc.vector.memset(ones[:], 1.0)
            for h in range(2):
                for j in range(2):
                    nc.gpsimd.dma_start(
                        Vt[h][:, ds(j * 32, 32), :],
                        v_all.ap()[h][:, ds(j * 32, 32), :].bitcast(dtr))
                nc.vector.tensor_copy(Vt[h][:, 64, :], ones[:])
            nc.sync.dma_start(kn_sb[:, :, :],
                              kn.ap().rearrange("(n p) f -> p n f", p=128))
            for i in range(1, 7):
                nc.sync.dma_start(KT[:, ds(i * 1024, 1024)],
                                  kt_old.ap()[:, ds(i * 1024, 1024)].bitcast(dtr))

            # ---- RoPE (interleaved pairs) on DVE ----
            def rope(src, dst, nsl, pfx):
                cvw = cos_sb[:, nsl, :].rearrange("p n (h j) -> p n h j", h=2)
                svw = sin_sb[:, nsl, :].rearrange("p n (h j) -> p n h j", h=2)
                x = src[:, nsl, :].rearrange("p n (h j two) -> p n h j two",
                                             h=2, two=2)
                o = dst[:, nsl, :].rearrange("p n (h j two) -> p n h j two",
                                             h=2, two=2)
                nb = x.shape[1]
                x1, x2 = x[:, :, :, :, 0], x[:, :, :, :, 1]
                o1, o2 = o[:, :, :, :, 0], o[:, :, :, :, 1]
                ta = wkp.tile([128, nb, 2, 32], dt, tag="rt", bufs=4,
                              name=f"{pfx}ta")
                tb = wkp.tile([128, nb, 2, 32], dt, tag="rt", bufs=4,
                              name=f"{pfx}tb")
                nc.vector.tensor_mul(ta[:], x1, cvw)
                nc.vector.tensor_mul(tb[:], x2, svw)
                nc.vector.tensor_sub(o1, ta[:], tb[:])
                tc2 = wkp.tile([128, nb, 2, 32], dt, tag="rt", bufs=4,
                               name=f"{pfx}tc")
                td = wkp.tile([128, nb, 2, 32], dt, tag="rt", bufs=4,
                               name=f"{pfx}td")
                nc.vector.tensor_mul(tc2[:], x2, cvw)
                nc.vector.tensor_mul(td[:], x1, svw)
                nc.vector.tensor_add(o2, tc2[:], td[:])

            # ---- rope + transpose: roped [tok, (h d)] -> [(h d), tok] ----
            # q first (QK needs QT immediately); k-new last (kb >= 56).
            def tposes(src, dstcols, rng, pool_tag_name):
                tp = stp.tile([128, 3, 512], dt, tag="st", bufs=2,
                              name=pool_tag_name)
                tv = tp.rearrange("p a b -> p (a b)")
                for i, n in enumerate(rng):
                    slot = tv[:, ds(i * 128, 128)]
                    nc.tensor.transpose(slot, src[:, n, :], ident[:, :])
                    nc.vector.tensor_copy(
                        dstcols[:, ds(n * 128, 128)], slot)

            rope(qn_sb, qr, ds(0, 4), "qa")
            tposes(qr, QT, range(0, 4), "tpqa")
            rope(qn_sb, qr, ds(4, 4), "qb")
            tposes(qr, QT, range(4, 8), "tpqb")
            rope(kn_sb, kr, ds(0, 8), "k")
            tposes(kr, KT[:, ds(OLD, S)], range(0, 8), "tpk")

            # ---- main loop: QK -> exp -> PV ----
            osb = {}
            for qt in range(2):
                qsl = ds(qt * 512, 512)
                pvt = [pvp.tile([65, 512], dt, tag=f"pv{h}", bufs=1,
                                name=f"pv{qt}{h}") for h in range(2)]
                if qt == 0:
                    slices = ([(kb, 0) for kb in range(12)]
                              + [(kb, 1) for kb in range(12)]
                              + [(kb, h) for kb in range(12, NKB)
                                 for h in range(2)])
                else:
                    slices = [(kb, h) for kb in range(NKB) for h in range(2)]
                for b0 in range(0, len(slices), 3):
                    batch = slices[b0:b0 + 3]
                    nb = len(batch)
                    st = stp.tile([128, 3, 512], dt, tag="st", bufs=2,
                                  name=f"st{qt}_{b0}")
                    for i, (kb, h) in enumerate(batch):
                        nc.tensor.matmul(
                            st[:, i, :],
                            lhsT=KT[64 * h:64 * h + 64, ds(kb * 128, 128)],
                            rhs=QT[64 * h:64 * h + 64, qsl],
                            start=True, stop=True,
                            tile_position=(64 * h, 0),
                        )
                    pt = ptp.tile([128, 3, 512], dtr, tag="pt", bufs=3,
                                  name=f"pt{qt}_{b0}")
                    nc.scalar.activation(pt[:, 0:nb, :], st[:, 0:nb, :],
                                         AF.Exp, scale=SCALE)
                    for i, (kb, h) in enumerate(batch):
                        nc.tensor.matmul(
                            pvt[h],
                            lhsT=Vt[h][:, :, kb],
                            rhs=pt[:, i, :],
                            start=(kb == 0), stop=(kb == NKB - 1),
                        )
                for h in range(2):
                    ot = osbp.tile([65, 512], dt, tag=f"osb{qt}{h}", bufs=1,
                                   name=f"osb{qt}{h}")
                    nc.vector.tensor_copy(ot[:], pvt[h])
                    osb[(qt, h)] = ot

            # ---- tail: transpose, normalize, store ----
            for qt in range(2):
                for h in range(2):
                    ot = osb[(qt, h)]
                    tt = stp.tile([128, 3, 512], dt, tag="st", bufs=2,
                                  name=f"tt{qt}{h}")
                    for j in range(4):
                        slot = tt[:, j % 3, ds((j // 3) * 128, 65)]
                        nc.tensor.transpose(slot, ot[:, ds(j * 128, 128)],
                                            ident[0:65, 0:65])
                        rec = finp.tile([128, 1], dt, tag="rec", bufs=4,
                                        name=f"rec{qt}{h}{j}")
                        nc.vector.reciprocal(rec[:], slot[:, 64:65])
                        fin = finp.tile([128, 64], dt, tag="fin", bufs=4,
                                        name=f"fin{qt}{h}{j}")
                        nc.vector.tensor_scalar(fin[:], slot[:, 0:64],
                                                rec[:], None, ALU.mult)
                        nc.sync.dma_start(
                            out.ap()[ds(qt * 512 + j * 128, 128), h, :],
                            fin[:])


def _prep_inputs(q, k, v, cache_k, cache_v, freqs_cos, freqs_sin):
    """Host-side sharding + layout prep (no FLOPs beyond data movement)."""
    q = np.asarray(q, np.float32)
    k = np.asarray(k, np.float32)
    v = np.asarray(v, np.float32)
    cache_k = np.asarray(cache_k, np.float32)
    cache_v = np.asarray(cache_v, np.float32)
    cos_h = np.asarray(freqs_cos, np.float32)[START:START + S, 0::2]
    sin_h = np.asarray(freqs_sin, np.float32)[START:START + S, 0::2]
    cs = np.ascontiguousarray(
        np.concatenate([np.tile(cos_h, (1, 2)), np.tile(sin_h, (1, 2))],
                       axis=1))                                 # [1024, 128]

    in_maps = []
    for c in range(NCORES):
        hs = slice(2 * c, 2 * c + 2)
        k_old = cache_k[0, CLO:CHI, hs, :]                      # [7168, 2, 64]
        kt_old = np.ascontiguousarray(
            k_old.transpose(1, 2, 0).reshape(128, OLD))
        # V window (old cache rows + raw new v), laid out [h, p, d, kb]
        vw = np.concatenate([cache_v[0, CLO:CHI, hs, :],
                             v[0, :, hs, :]], axis=0)           # [8192, 2, 64]
        v_all = np.ascontiguousarray(
            vw.reshape(NKB, 128, 2, D).transpose(2, 1, 3, 0))   # [2,128,64,64]
        qn = np.ascontiguousarray(q[0, :, hs, :].reshape(S, 128))
        kn = np.ascontiguousarray(k[0, :, hs, :].reshape(S, 128))
        in_maps.append({
            "kt_old": kt_old, "v_all": v_all, "qn": qn, "kn": kn, "cs": cs,
        })
    return in_maps


def get_nc(niters=1):
    key = ("nc", niters)
    if key not in _cache:
        _cache[key] = _build(niters)
    return _cache[key]


def _run(in_maps, niters=1):
    from concourse.bass_utils import run_bass_kernel_spmd
    res = run_bass_kernel_spmd(get_nc(niters), in_maps,
                               core_ids=list(range(NCORES)))
    out_full = np.empty((1, S, H, D), np.float32)
    for c in range(NCORES):
        out_full[0, :, 2 * c:2 * c + 2, :] = res.results[c]["out"].reshape(
            S, 2, D)
    return out_full.reshape(1, S, H * D), res


def kernel(q, k, v, cache_k, cache_v, freqs_cos, freqs_sin):
    in_maps = _prep_inputs(q, k, v, cache_k, cache_v, freqs_cos, freqs_sin)
    out, _ = _run(in_maps)
    return out
